# revision 8
# baseline (speedup 1.0000x reference)
"""GQA attention (32 Q heads / 8 KV heads, S=2048, D=4096, rotate-half RoPE,
causal sliding-window mask with window >= S) on 8 Trainium2 NeuronCores.

Sharding: tensor-parallel over heads. Core c owns Q heads 4c..4c+3 and KV head
c (one full GQA group): Wq/Wk/Wv column-sharded, Wo row-sharded. x replicated
(passed pre-transposed as xT so the contraction dim lands on partitions). Each
core returns a partial out^T [4096, 2048]; the host sums the 8 partials
(all-reduce) and transposes back. k/v windows come back per-core already in
natural [token, dim] layout.

Device-side dataflow per core:
  phase 1: qT/kT/vT = (W^T x^T) via PSUM-accumulated f32r matmuls, K split in
           two halves so each half's weights fit SBUF; RoPE applied in the
           transposed layout using a partition-swapped copy (DVE lanes are
           partition-locked) with host-built cosD/sinD carrying the signs.
  phase 2: scores^T = kT.T @ qT per (head, 128-wide k-tile, 512-wide q-block)
           -> exp on ACT straight out of PSUM into bf16 P^T tiles -> causal
           masking by multiplying the 4 diagonal tiles with a binary mask ->
           P.V via bf16 matmuls against V augmented with a ones column (the
           ones column accumulates the softmax denominator, so no partition
           reduction is ever needed) -> normalize by the reciprocal column
           (per-partition tensor_scalar) -> PE-transpose -> attn^T.
  phase 3: out^T += Wo^T attn^T per 512-token block, DMA'd to DRAM from PSUM.
"""

import sys

sys.path.insert(0, "/opt/trn_rl_repo")

from contextlib import ExitStack

import numpy as np
import ml_dtypes

import concourse.bass as bass
import concourse.mybir as mybir
from concourse import bacc
from concourse.bass import ts
from concourse.bass_utils import run_bass_kernel_spmd
from concourse.tile import TileContext

F32 = mybir.dt.float32
F32R = mybir.dt.float32r
BF16 = mybir.dt.bfloat16

D_MODEL = 4096
S = 2048
HEAD_DIM = 128
NUM_HEADS = 32
NUM_KV_HEADS = 8
N_CORES = 8
HQ = NUM_HEADS // N_CORES  # 4 q heads per core
QW = HQ * HEAD_DIM  # 512 q columns per core
SCALE = 1.0 / np.sqrt(HEAD_DIM)
NAUG = 132  # v columns (128) + ones column + pad
NB = 4  # token blocks of 512
BLK = S // NB

TRACE = False
TRACE_DIR = None
LAST_RESULT = None

_PROG = None


def _build():
    nc = bacc.Bacc("TRN2", target_bir_lowering=False, debug=False, num_devices=N_CORES)

    xt = nc.dram_tensor("xt", [D_MODEL, S], F32R, kind="ExternalInput").ap()
    wq = nc.dram_tensor("wq", [D_MODEL, QW], F32R, kind="ExternalInput").ap()
    wk = nc.dram_tensor("wk", [D_MODEL, HEAD_DIM], F32R, kind="ExternalInput").ap()
    wv = nc.dram_tensor("wv", [D_MODEL, HEAD_DIM], F32R, kind="ExternalInput").ap()
    wo = nc.dram_tensor("wo", [QW, D_MODEL], F32R, kind="ExternalInput").ap()
    cosd = nc.dram_tensor("cosd", [128, S], F32, kind="ExternalInput").ap()
    sind = nc.dram_tensor("sind", [128, S], F32, kind="ExternalInput").ap()
    maskb = nc.dram_tensor("maskb", [128, 896], BF16, kind="ExternalInput").ap()
    idf = nc.dram_tensor("idf", [128, 128], F32, kind="ExternalInput").ap()
    idr = nc.dram_tensor("idr", [128, 128], F32R, kind="ExternalInput").ap()

    outt = nc.dram_tensor("outt", [D_MODEL, S], F32, kind="ExternalOutput").ap()
    ko = nc.dram_tensor("ko", [S, HEAD_DIM], F32, kind="ExternalOutput").ap()
    vo = nc.dram_tensor("vo", [S, HEAD_DIM], F32, kind="ExternalOutput").ap()

    EXP = mybir.ActivationFunctionType.Exp

    with TileContext(nc) as tc, ExitStack() as top:
        persist = top.enter_context(tc.tile_pool(name="persist", bufs=1))

        cosd_sb = persist.tile([128, S], F32)
        nc.sync.dma_start(out=cosd_sb[:, :], in_=cosd[:, :])
        sind_sb = persist.tile([128, S], F32)
        nc.sync.dma_start(out=sind_sb[:, :], in_=sind[:, :])
        mask_sb = persist.tile([128, 896], BF16)
        nc.sync.dma_start(out=mask_sb[:, :], in_=maskb[:, :])
        ident = persist.tile([128, 128], F32)
        nc.sync.dma_start(out=ident[:, :], in_=idf[:, :])
        identr = persist.tile([128, 128], F32R)
        nc.sync.dma_start(out=identr[:, :], in_=idr[:, :])

        qrot = persist.tile([128, HQ, S], F32R)
        krot = persist.tile([128, S], F32R)
        vaug = persist.tile([128, S // 128, NAUG], BF16)
        nc.vector.memset(vaug[:, :, 128:129], 1.0)
        nc.vector.memset(vaug[:, :, 129:NAUG], 0.0)

        # ---------------- phase 1: projections + RoPE ----------------
        with ExitStack() as p1:
            wgt = p1.enter_context(tc.tile_pool(name="wgt", bufs=1))
            acc = p1.enter_context(tc.tile_pool(name="acc", bufs=1))
            xtp = p1.enter_context(tc.tile_pool(name="xtp", bufs=4))
            swp = p1.enter_context(tc.tile_pool(name="swp", bufs=2))
            rtmp = p1.enter_context(tc.tile_pool(name="rtmp", bufs=2))
            pq = p1.enter_context(tc.tile_pool(name="pq", bufs=1, space="PSUM"))
            pt2 = p1.enter_context(tc.tile_pool(name="pt2", bufs=2, space="PSUM"))

            qsb = acc.tile([128, HQ, S], F32, tag="qsb")
            ksb = acc.tile([128, S], F32, tag="ksb")
            vsb = acc.tile([128, S], F32, tag="vsb")

            def rope_full(src, dst, bsl):
                # dst = src*cosD + swap_halves(src)*sinD
                sw = swp.tile([128, BLK], F32, tag="sw")
                nc.sync.dma_start(out=sw[0:64, :], in_=src[64:128, :])
                nc.sync.dma_start(out=sw[64:128, :], in_=src[0:64, :])
                tm = rtmp.tile([128, BLK], F32, tag="rt")
                nc.vector.tensor_mul(dst, src, cosd_sb[:, bsl])
                nc.vector.tensor_mul(tm[:, :], sw[:, :], sind_sb[:, bsl])
                nc.vector.tensor_add(dst, dst, tm[:, :])

            for half in range(2):
                hrows = slice(half * 2048, (half + 1) * 2048)
                wq_sb = wgt.tile([128, 16, QW], F32R, tag="wq")
                nc.sync.dma_start(
                    out=wq_sb[:, :, :],
                    in_=wq[hrows, :].rearrange("(c p) m -> p c m", p=128),
                )
                wk_sb = wgt.tile([128, 16, HEAD_DIM], F32R, tag="wk")
                nc.sync.dma_start(
                    out=wk_sb[:, :, :],
                    in_=wk[hrows, :].rearrange("(c p) m -> p c m", p=128),
                )
                wv_sb = wgt.tile([128, 16, HEAD_DIM], F32R, tag="wv")
                nc.sync.dma_start(
                    out=wv_sb[:, :, :],
                    in_=wv[hrows, :].rearrange("(c p) m -> p c m", p=128),
                )

                for b in range(NB):
                    bsl = slice(b * BLK, (b + 1) * BLK)
                    qp = pq.tile([128, HQ, BLK], F32, tag="qp")
                    kp = pq.tile([128, BLK], F32, tag="kp")
                    vp = pq.tile([128, BLK], F32, tag="vp")
                    for c in range(16):
                        xt_t = xtp.tile([128, BLK], F32R, tag="xt")
                        r0 = half * 2048 + c * 128
                        nc.sync.dma_start(
                            out=xt_t[:, :], in_=xt[r0 : r0 + 128, bsl]
                        )
                        xr = xt_t[:, :]
                        st, sp = (c == 0), (c == 15)
                        for m in range(HQ):
                            nc.tensor.matmul(
                                qp[:, m, :],
                                wq_sb[:, c, ts(m, 128)],
                                xr,
                                start=st,
                                stop=sp,
                            )
                        nc.tensor.matmul(
                            kp[:, :], wk_sb[:, c, :], xr,
                            start=st, stop=sp,
                        )
                        nc.tensor.matmul(
                            vp[:, :], wv_sb[:, c, :], xr,
                            start=st, stop=sp,
                        )
                    if half == 0:
                        for m in range(HQ):
                            nc.vector.tensor_copy(out=qsb[:, m, bsl], in_=qp[:, m, :])
                        nc.vector.tensor_copy(out=ksb[:, bsl], in_=kp[:, :])
                        nc.vector.tensor_copy(out=vsb[:, bsl], in_=vp[:, :])
                    else:
                        for m in range(HQ):
                            nc.vector.tensor_add(qsb[:, m, bsl], qp[:, m, :], qsb[:, m, bsl])
                        nc.vector.tensor_add(ksb[:, bsl], kp[:, :], ksb[:, bsl])
                        nc.vector.tensor_add(vsb[:, bsl], vp[:, :], vsb[:, bsl])

                        for m in range(HQ):
                            rope_full(qsb[:, m, bsl], qrot[:, m, bsl], bsl)
                        rope_full(ksb[:, bsl], krot[:, bsl], bsl)

                        for t in range(4 * b, 4 * b + 4):
                            ktp = pt2.tile([128, 128], F32R, tag="tp")
                            nc.tensor.transpose(ktp[:, :], krot[:, ts(t, 128)], identr[:, :])
                            kst = swp.tile([128, 128], F32, tag="kst")
                            nc.scalar.copy(out=kst[:, :], in_=ktp[:, :])
                            nc.sync.dma_start(out=ko[ts(t, 128), :], in_=kst[:, :])
                            vtp = pt2.tile([128, 128], F32, tag="tp")
                            nc.tensor.transpose(vtp[:, :], vsb[:, ts(t, 128)], ident[:, :])
                            vst = swp.tile([128, 128], F32, tag="vst")
                            nc.scalar.copy(out=vst[:, :], in_=vtp[:, :])
                            nc.sync.dma_start(out=vo[ts(t, 128), :], in_=vst[:, :])
                            nc.vector.tensor_copy(out=vaug[:, t, 0:128], in_=vtp[:, :])

        # ---------------- phase 2/3: attention + output projection ----------------
        with ExitStack() as p2:
            w2 = p2.enter_context(tc.tile_pool(name="w2", bufs=1))
            osp = p2.enter_context(tc.tile_pool(name="osp", bufs=4))
            atbp = p2.enter_context(tc.tile_pool(name="atbp", bufs=2))
            ptp = p2.enter_context(tc.tile_pool(name="ptp", bufs=20))
            anp = p2.enter_context(tc.tile_pool(name="anp", bufs=3))
            rcp = p2.enter_context(tc.tile_pool(name="rcp", bufs=3))
            ps_st = p2.enter_context(tc.tile_pool(name="ps_st", bufs=2, space="PSUM"))
            ps_ap = p2.enter_context(tc.tile_pool(name="ps_ap", bufs=2, space="PSUM"))
            ps_at = p2.enter_context(tc.tile_pool(name="ps_at", bufs=2, space="PSUM"))
            ps_op = p2.enter_context(tc.tile_pool(name="ps_op", bufs=2, space="PSUM"))

            wo_sb = w2.tile([128, HQ, D_MODEL], F32R)
            wo_r = wo.rearrange("(h p) o -> p h o", p=128)
            for i in range(4):
                osl = slice(i * 1024, (i + 1) * 1024)
                nc.sync.dma_start(out=wo_sb[:, :, osl], in_=wo_r[:, :, osl])

            for b in range(NB):
                bsl = slice(b * BLK, (b + 1) * BLK)
                atb = atbp.tile([128, HQ, BLK], F32R, tag="atb")
                for h in range(HQ):
                    pts = []
                    for t in range(4 * b + 4):
                        st_ = ps_st.tile([128, BLK], F32, tag="st")
                        nc.tensor.matmul(
                            st_[:, :],
                            krot[:, ts(t, 128)],
                            qrot[:, h, bsl],
                            start=True,
                            stop=True,
                        )
                        pt_ = ptp.tile([128, BLK], BF16, tag="pt")
                        nc.scalar.activation(pt_[:, :], st_[:, :], EXP)
                        if t >= 4 * b:
                            j = t - 4 * b
                            pm = ptp.tile([128, BLK], BF16, tag="pt")
                            ms = 384 - j * 128
                            nc.vector.tensor_mul(
                                pm[:, :], pt_[:, :], mask_sb[:, ms : ms + BLK]
                            )
                            pt_ = pm
                        pts.append(pt_)
                    for tau in range(4):
                        T = 4 * b + tau
                        ap_ = ps_ap.tile([128, NAUG], F32, tag="ap")
                        for t in range(T + 1):
                            nc.tensor.matmul(
                                ap_[:, :],
                                pts[t][:, ts(tau, 128)],
                                vaug[:, t, :],
                                start=(t == 0),
                                stop=(t == T),
                            )
                        rc = rcp.tile([128, 1], F32, tag="rc")
                        nc.vector.reciprocal(rc[:, :], ap_[:, 128:129])
                        an = anp.tile([128, 128], F32R, tag="an")
                        nc.vector.tensor_scalar_mul(an[:, :], ap_[:, 0:128], rc[:, :])
                        atp_ = ps_at.tile([128, 128], F32R, tag="at")
                        nc.tensor.transpose(atp_[:, :], an[:, :], identr[:, :])
                        nc.vector.tensor_copy(out=atb[:, h, ts(tau, 128)], in_=atp_[:, :])
                for o in range(32):
                    op = ps_op.tile([128, BLK], F32, tag="op")
                    for h in range(HQ):
                        nc.tensor.matmul(
                            op[:, :],
                            wo_sb[:, h, ts(o, 128)],
                            atb[:, h, :],
                            start=(h == 0),
                            stop=(h == HQ - 1),
                        )
                    os_ = osp.tile([128, BLK], F32, tag="os")
                    # alternate PSUM->SBUF copies between ACT and DVE so
                    # neither engine becomes the drain bottleneck
                    if o % 2 == 0:
                        nc.scalar.copy(out=os_[:, :], in_=op[:, :])
                    else:
                        nc.vector.tensor_copy(out=os_[:, :], in_=op[:, :])
                    nc.sync.dma_start(out=outt[ts(o, 128), bsl], in_=os_[:, :])

    nc.compile()
    return nc


def _get_prog():
    global _PROG
    if _PROG is None:
        _PROG = _build()
    return _PROG


def kernel(x, Wq, Wk, Wv, Wo, offset):
    x = np.asarray(x, dtype=np.float32)
    Wq = np.asarray(Wq, dtype=np.float32)
    Wk = np.asarray(Wk, dtype=np.float32)
    Wv = np.asarray(Wv, dtype=np.float32)
    Wo = np.asarray(Wo, dtype=np.float32)
    off = int(np.asarray(offset).reshape(()))

    B = x.shape[0]
    assert x.shape == (B, S, D_MODEL) and B == 1

    xt = np.ascontiguousarray(x[0].T)  # [D, S]

    half = HEAD_DIM // 2
    inv_freq = 1.0 / (10000.0 ** (np.arange(0, half, dtype=np.float64) / half))
    pos = np.arange(S, dtype=np.float64) + off
    ang = pos[:, None] * inv_freq[None, :]  # [S, 64]
    cos_t = np.cos(ang).T.astype(np.float32)  # [64, S]
    sin_t = np.sin(ang).T.astype(np.float32)
    cosd = np.ascontiguousarray(np.concatenate([cos_t, cos_t], axis=0))  # [128, S]
    sind = np.ascontiguousarray(np.concatenate([-sin_t, sin_t], axis=0))

    p = np.arange(128)[:, None]
    c = np.arange(896)[None, :]
    maskb = (c >= p + 384).astype(ml_dtypes.bfloat16)

    wq_s = (Wq * SCALE).astype(np.float32)
    in_maps = []
    for cix in range(N_CORES):
        in_maps.append(
            {
                "xt": xt,
                "wq": np.ascontiguousarray(wq_s[:, cix * QW : (cix + 1) * QW]),
                "wk": np.ascontiguousarray(Wk[:, cix * HEAD_DIM : (cix + 1) * HEAD_DIM]),
                "wv": np.ascontiguousarray(Wv[:, cix * HEAD_DIM : (cix + 1) * HEAD_DIM]),
                "wo": np.ascontiguousarray(Wo[cix * QW : (cix + 1) * QW, :]),
                "cosd": cosd,
                "sind": sind,
                "maskb": maskb,
                "idf": np.eye(128, dtype=np.float32),
                "idr": np.eye(128, dtype=np.float32),
            }
        )

    nc = _get_prog()
    kwargs = {}
    if TRACE:
        kwargs = dict(trace=True)
        if TRACE_DIR:
            kwargs["tmpdir"] = TRACE_DIR
    res = run_bass_kernel_spmd(nc, in_maps, list(range(N_CORES)), **kwargs)
    global LAST_RESULT
    LAST_RESULT = res

    outt = np.zeros((D_MODEL, S), dtype=np.float32)
    for cix in range(N_CORES):
        outt += res.results[cix]["outt"]
    out = np.ascontiguousarray(outt.T)[None]  # [1, S, D]

    k_window = np.stack([res.results[cix]["ko"] for cix in range(N_CORES)])[None]
    v_window = np.stack([res.results[cix]["vo"] for cix in range(N_CORES)])[None]
    return out, k_window, v_window


# revision 11
# speedup vs baseline: 1.0251x; 1.0251x over previous
"""GQA attention (32 Q heads / 8 KV heads, S=2048, D=4096, rotate-half RoPE,
causal sliding-window mask with window >= S) on 8 Trainium2 NeuronCores.

Sharding: tensor-parallel over heads. Core c owns Q heads 4c..4c+3 and KV head
c (one full GQA group): Wq/Wk/Wv column-sharded, Wo row-sharded. x replicated
(passed pre-transposed as xT so the contraction dim lands on partitions). Each
core returns a partial out^T [4096, 2048]; the host sums the 8 partials
(all-reduce) and transposes back. k/v windows come back per-core already in
natural [token, dim] layout.

Device-side dataflow per core:
  phase 1: qT/kT/vT = (W^T x^T) via PSUM-accumulated f32r matmuls, K split in
           two halves so each half's weights fit SBUF; RoPE applied in the
           transposed layout using a partition-swapped copy (DVE lanes are
           partition-locked) with host-built cosD/sinD carrying the signs.
  phase 2: scores^T = kT.T @ qT per (head, 128-wide k-tile, 512-wide q-block)
           -> exp on ACT straight out of PSUM into bf16 P^T tiles -> causal
           masking by multiplying the 4 diagonal tiles with a binary mask ->
           P.V via bf16 matmuls against V augmented with a ones column (the
           ones column accumulates the softmax denominator, so no partition
           reduction is ever needed) -> normalize by the reciprocal column
           (per-partition tensor_scalar) -> PE-transpose -> attn^T.
  phase 3: out^T += Wo^T attn^T per 512-token block, DMA'd to DRAM from PSUM.
"""

import sys

sys.path.insert(0, "/opt/trn_rl_repo")

from contextlib import ExitStack

import numpy as np
import ml_dtypes

import concourse.bass as bass
import concourse.mybir as mybir
from concourse import bacc
from concourse.bass import ts
from concourse.bass_utils import run_bass_kernel_spmd
from concourse.tile import TileContext

F32 = mybir.dt.float32
F32R = mybir.dt.float32r
BF16 = mybir.dt.bfloat16

D_MODEL = 4096
S = 2048
HEAD_DIM = 128
NUM_HEADS = 32
NUM_KV_HEADS = 8
N_CORES = 8
HQ = NUM_HEADS // N_CORES  # 4 q heads per core
QW = HQ * HEAD_DIM  # 512 q columns per core
SCALE = 1.0 / np.sqrt(HEAD_DIM)
NAUG = 132  # v columns (128) + ones column + pad
NB = 4  # token blocks of 512
BLK = S // NB

TRACE = False
TRACE_DIR = None
LAST_RESULT = None

_PROG = None


def _build():
    nc = bacc.Bacc("TRN2", target_bir_lowering=False, debug=False, num_devices=N_CORES)

    xt = nc.dram_tensor("xt", [D_MODEL, S], F32R, kind="ExternalInput").ap()
    wq = nc.dram_tensor("wq", [D_MODEL, QW], F32R, kind="ExternalInput").ap()
    wk = nc.dram_tensor("wk", [D_MODEL, HEAD_DIM], F32R, kind="ExternalInput").ap()
    wv = nc.dram_tensor("wv", [D_MODEL, HEAD_DIM], F32R, kind="ExternalInput").ap()
    wo = nc.dram_tensor("wo", [QW, D_MODEL], F32R, kind="ExternalInput").ap()
    cosd = nc.dram_tensor("cosd", [128, S], F32, kind="ExternalInput").ap()
    sind = nc.dram_tensor("sind", [128, S], F32, kind="ExternalInput").ap()
    maskb = nc.dram_tensor("maskb", [128, 896], BF16, kind="ExternalInput").ap()
    idf = nc.dram_tensor("idf", [128, 128], F32, kind="ExternalInput").ap()
    idr = nc.dram_tensor("idr", [128, 128], F32R, kind="ExternalInput").ap()

    outt = nc.dram_tensor("outt", [D_MODEL, S], F32, kind="ExternalOutput").ap()
    ko = nc.dram_tensor("ko", [S, HEAD_DIM], F32, kind="ExternalOutput").ap()
    vo = nc.dram_tensor("vo", [S, HEAD_DIM], F32, kind="ExternalOutput").ap()

    EXP = mybir.ActivationFunctionType.Exp

    with TileContext(nc) as tc, ExitStack() as top:
        persist = top.enter_context(tc.tile_pool(name="persist", bufs=1))

        mask_sb = persist.tile([128, 896], BF16)
        nc.sync.dma_start(out=mask_sb[:, :], in_=maskb[:, :])
        identr = persist.tile([128, 128], F32R)
        nc.sync.dma_start(out=identr[:, :], in_=idr[:, :])

        qrot = persist.tile([128, HQ, S], F32R)
        krot = persist.tile([128, S], F32R)
        vaug = persist.tile([128, S // 128, NAUG], BF16)
        nc.vector.memset(vaug[:, :, 128:129], 1.0)
        nc.vector.memset(vaug[:, :, 129:NAUG], 0.0)

        # ---------------- phase 1: projections + RoPE ----------------
        with ExitStack() as p1:
            wgt = p1.enter_context(tc.tile_pool(name="wgt", bufs=1))
            p1const = p1.enter_context(tc.tile_pool(name="p1const", bufs=1))
            acc = p1.enter_context(tc.tile_pool(name="acc", bufs=1))
            xtp = p1.enter_context(tc.tile_pool(name="xtp", bufs=4))
            swp = p1.enter_context(tc.tile_pool(name="swp", bufs=2))
            rtmp = p1.enter_context(tc.tile_pool(name="rtmp", bufs=2))
            pq = p1.enter_context(tc.tile_pool(name="pq", bufs=1, space="PSUM"))
            pkv = p1.enter_context(tc.tile_pool(name="pkv", bufs=1, space="PSUM"))
            pt2 = p1.enter_context(tc.tile_pool(name="pt2", bufs=2, space="PSUM"))

            qsb = acc.tile([128, HQ, S], F32, tag="qsb")
            ksb = acc.tile([128, S], F32, tag="ksb")
            vsb = acc.tile([128, S], F32, tag="vsb")

            def rope_full(src, dst, bsl):
                # dst = src*cosD + swap_halves(src)*sinD
                sw = swp.tile([128, BLK], F32, tag="sw")
                nc.sync.dma_start(out=sw[0:64, :], in_=src[64:128, :])
                nc.sync.dma_start(out=sw[64:128, :], in_=src[0:64, :])
                tm = rtmp.tile([128, BLK], F32, tag="rt")
                nc.vector.tensor_mul(dst, src, cosd_sb[:, bsl])
                nc.vector.tensor_mul(tm[:, :], sw[:, :], sind_sb[:, bsl])
                nc.vector.tensor_add(dst, dst, tm[:, :])

            cosd_sb = None
            for half in range(2):
                wqs, wks, wvs = [], [], []
                for c in range(16):
                    r0 = half * 2048 + c * 128
                    wqt = wgt.tile([128, QW], F32R, tag=f"wq{c}")
                    nc.sync.dma_start(out=wqt[:, :], in_=wq[r0 : r0 + 128, :])
                    wkt = wgt.tile([128, HEAD_DIM], F32R, tag=f"wk{c}")
                    nc.sync.dma_start(out=wkt[:, :], in_=wk[r0 : r0 + 128, :])
                    wvt = wgt.tile([128, HEAD_DIM], F32R, tag=f"wv{c}")
                    nc.sync.dma_start(out=wvt[:, :], in_=wv[r0 : r0 + 128, :])
                    wqs.append(wqt)
                    wks.append(wkt)
                    wvs.append(wvt)
                if cosd_sb is None:
                    # loaded after the first weight chunks so the first
                    # matmuls aren't queued behind 2MB of RoPE tables
                    cosd_sb = p1const.tile([128, S], F32)
                    nc.sync.dma_start(out=cosd_sb[:, :], in_=cosd[:, :])
                    sind_sb = p1const.tile([128, S], F32)
                    nc.sync.dma_start(out=sind_sb[:, :], in_=sind[:, :])
                    ident = p1const.tile([128, 128], F32)
                    nc.sync.dma_start(out=ident[:, :], in_=idf[:, :])

                for b in range(NB):
                    bsl = slice(b * BLK, (b + 1) * BLK)
                    qp = pq.tile([128, HQ, BLK], F32, tag="qp")
                    kp = pkv.tile([128, BLK], F32, tag="kp")
                    vp = pkv.tile([128, BLK], F32, tag="vp")
                    for c in range(16):
                        xt_t = xtp.tile([128, BLK], F32R, tag="xt")
                        r0 = half * 2048 + c * 128
                        nc.sync.dma_start(
                            out=xt_t[:, :], in_=xt[r0 : r0 + 128, bsl]
                        )
                        xr = xt_t[:, :]
                        st, sp = (c == 0), (c == 15)
                        for m in range(HQ):
                            nc.tensor.matmul(
                                qp[:, m, :],
                                wqs[c][:, ts(m, 128)],
                                xr,
                                start=st,
                                stop=sp,
                            )
                        nc.tensor.matmul(
                            kp[:, :], wks[c][:, :], xr,
                            start=st, stop=sp,
                        )
                        nc.tensor.matmul(
                            vp[:, :], wvs[c][:, :], xr,
                            start=st, stop=sp,
                        )
                    if half == 0:
                        for m in range(HQ):
                            nc.vector.tensor_copy(out=qsb[:, m, bsl], in_=qp[:, m, :])
                        nc.vector.tensor_copy(out=ksb[:, bsl], in_=kp[:, :])
                        nc.vector.tensor_copy(out=vsb[:, bsl], in_=vp[:, :])
                    else:
                        for m in range(HQ):
                            nc.vector.tensor_add(qsb[:, m, bsl], qp[:, m, :], qsb[:, m, bsl])
                        nc.vector.tensor_add(ksb[:, bsl], kp[:, :], ksb[:, bsl])
                        nc.vector.tensor_add(vsb[:, bsl], vp[:, :], vsb[:, bsl])

                        for m in range(HQ):
                            rope_full(qsb[:, m, bsl], qrot[:, m, bsl], bsl)
                        rope_full(ksb[:, bsl], krot[:, bsl], bsl)

                        for t in range(4 * b, 4 * b + 4):
                            ktp = pt2.tile([128, 128], F32R, tag="tp")
                            nc.tensor.transpose(ktp[:, :], krot[:, ts(t, 128)], identr[:, :])
                            kst = swp.tile([128, 128], F32, tag="kst")
                            nc.scalar.copy(out=kst[:, :], in_=ktp[:, :])
                            nc.sync.dma_start(out=ko[ts(t, 128), :], in_=kst[:, :])
                            vtp = pt2.tile([128, 128], F32, tag="tp")
                            nc.tensor.transpose(vtp[:, :], vsb[:, ts(t, 128)], ident[:, :])
                            vst = swp.tile([128, 128], F32, tag="vst")
                            nc.scalar.copy(out=vst[:, :], in_=vtp[:, :])
                            nc.sync.dma_start(out=vo[ts(t, 128), :], in_=vst[:, :])
                            nc.vector.tensor_copy(out=vaug[:, t, 0:128], in_=vtp[:, :])

        # ---------------- phase 2/3: attention + output projection ----------------
        with ExitStack() as p2:
            w2 = p2.enter_context(tc.tile_pool(name="w2", bufs=1))
            osp = p2.enter_context(tc.tile_pool(name="osp", bufs=4))
            atbp = p2.enter_context(tc.tile_pool(name="atbp", bufs=2))
            ptp = p2.enter_context(tc.tile_pool(name="ptp", bufs=34))
            anp = p2.enter_context(tc.tile_pool(name="anp", bufs=3))
            rcp = p2.enter_context(tc.tile_pool(name="rcp", bufs=3))
            ps_st = p2.enter_context(tc.tile_pool(name="ps_st", bufs=3, space="PSUM"))
            ps_ap = p2.enter_context(tc.tile_pool(name="ps_ap", bufs=2, space="PSUM"))
            ps_at = p2.enter_context(tc.tile_pool(name="ps_at", bufs=1, space="PSUM"))
            ps_op = p2.enter_context(tc.tile_pool(name="ps_op", bufs=2, space="PSUM"))

            # wo in 4 independent tiles so the first loads as soon as 2MB of
            # phase-1 SBUF frees, instead of waiting for a full 8MB hole
            wo_r = wo.rearrange("(h p) o -> p h o", p=128)
            wo_sb = []
            for i in range(4):
                osl = slice(i * 1024, (i + 1) * 1024)
                wt = w2.tile([128, HQ, 1024], F32R, tag=f"wo{i}")
                nc.sync.dma_start(out=wt[:, :, :], in_=wo_r[:, :, osl])
                wo_sb.append(wt)

            def qk_head(b, h):
                """scores^T -> exp -> masked bf16 P^T tiles for one head."""
                bsl = slice(b * BLK, (b + 1) * BLK)
                pts = []
                for t in range(4 * b + 4):
                    st_ = ps_st.tile([128, BLK], F32, tag="st")
                    nc.tensor.matmul(
                        st_[:, :],
                        krot[:, ts(t, 128)],
                        qrot[:, h, bsl],
                        start=True,
                        stop=True,
                    )
                    pt_ = ptp.tile([128, BLK], BF16, tag="pt")
                    nc.scalar.activation(pt_[:, :], st_[:, :], EXP)
                    if t >= 4 * b:
                        j = t - 4 * b
                        ms = 384 - j * 128
                        nc.vector.tensor_mul(
                            pt_[:, :], pt_[:, :], mask_sb[:, ms : ms + BLK]
                        )
                    pts.append(pt_)
                return pts

            for b in range(NB):
                bsl = slice(b * BLK, (b + 1) * BLK)
                atb = atbp.tile([128, HQ, BLK], F32R, tag="atb")
                pts_cur = qk_head(b, 0)
                for h in range(HQ):
                    # queue next head's QK ahead of this head's PV so the PE
                    # has work while ACT finishes this head's exp tiles
                    pts_next = qk_head(b, h + 1) if h < HQ - 1 else None
                    for tau in range(4):
                        T = 4 * b + tau
                        ap_ = ps_ap.tile([128, NAUG], F32, tag="ap")
                        for t in range(T + 1):
                            nc.tensor.matmul(
                                ap_[:, :],
                                pts_cur[t][:, ts(tau, 128)],
                                vaug[:, t, :],
                                start=(t == 0),
                                stop=(t == T),
                            )
                        rc = rcp.tile([128, 1], F32, tag="rc")
                        nc.vector.reciprocal(rc[:, :], ap_[:, 128:129])
                        an = anp.tile([128, 128], F32R, tag="an")
                        nc.vector.tensor_scalar_mul(an[:, :], ap_[:, 0:128], rc[:, :])
                        atp_ = ps_at.tile([128, 128], F32R, tag="at")
                        nc.tensor.transpose(atp_[:, :], an[:, :], identr[:, :])
                        nc.vector.tensor_copy(out=atb[:, h, ts(tau, 128)], in_=atp_[:, :])
                    pts_cur = pts_next
                for o in range(32):
                    op = ps_op.tile([128, BLK], F32, tag="op")
                    for h in range(HQ):
                        nc.tensor.matmul(
                            op[:, :],
                            wo_sb[o // 8][:, h, ts(o % 8, 128)],
                            atb[:, h, :],
                            start=(h == 0),
                            stop=(h == HQ - 1),
                        )
                    os_ = osp.tile([128, BLK], F32, tag="os")
                    # alternate PSUM->SBUF copies between ACT and DVE so
                    # neither engine becomes the drain bottleneck
                    if o % 2 == 0:
                        nc.scalar.copy(out=os_[:, :], in_=op[:, :])
                    else:
                        nc.vector.tensor_copy(out=os_[:, :], in_=op[:, :])
                    nc.sync.dma_start(out=outt[ts(o, 128), bsl], in_=os_[:, :])

    nc.compile()
    return nc


def _get_prog():
    global _PROG
    if _PROG is None:
        _PROG = _build()
    return _PROG


def kernel(x, Wq, Wk, Wv, Wo, offset):
    x = np.asarray(x, dtype=np.float32)
    Wq = np.asarray(Wq, dtype=np.float32)
    Wk = np.asarray(Wk, dtype=np.float32)
    Wv = np.asarray(Wv, dtype=np.float32)
    Wo = np.asarray(Wo, dtype=np.float32)
    off = int(np.asarray(offset).reshape(()))

    B = x.shape[0]
    assert x.shape == (B, S, D_MODEL) and B == 1

    xt = np.ascontiguousarray(x[0].T)  # [D, S]

    half = HEAD_DIM // 2
    inv_freq = 1.0 / (10000.0 ** (np.arange(0, half, dtype=np.float64) / half))
    pos = np.arange(S, dtype=np.float64) + off
    ang = pos[:, None] * inv_freq[None, :]  # [S, 64]
    cos_t = np.cos(ang).T.astype(np.float32)  # [64, S]
    sin_t = np.sin(ang).T.astype(np.float32)
    cosd = np.ascontiguousarray(np.concatenate([cos_t, cos_t], axis=0))  # [128, S]
    sind = np.ascontiguousarray(np.concatenate([-sin_t, sin_t], axis=0))

    p = np.arange(128)[:, None]
    c = np.arange(896)[None, :]
    maskb = (c >= p + 384).astype(ml_dtypes.bfloat16)

    wq_s = (Wq * SCALE).astype(np.float32)
    in_maps = []
    for cix in range(N_CORES):
        in_maps.append(
            {
                "xt": xt,
                "wq": np.ascontiguousarray(wq_s[:, cix * QW : (cix + 1) * QW]),
                "wk": np.ascontiguousarray(Wk[:, cix * HEAD_DIM : (cix + 1) * HEAD_DIM]),
                "wv": np.ascontiguousarray(Wv[:, cix * HEAD_DIM : (cix + 1) * HEAD_DIM]),
                "wo": np.ascontiguousarray(Wo[cix * QW : (cix + 1) * QW, :]),
                "cosd": cosd,
                "sind": sind,
                "maskb": maskb,
                "idf": np.eye(128, dtype=np.float32),
                "idr": np.eye(128, dtype=np.float32),
            }
        )

    nc = _get_prog()
    kwargs = {}
    if TRACE:
        kwargs = dict(trace=True)
        if TRACE_DIR:
            kwargs["tmpdir"] = TRACE_DIR
    res = run_bass_kernel_spmd(nc, in_maps, list(range(N_CORES)), **kwargs)
    global LAST_RESULT
    LAST_RESULT = res

    outt = np.zeros((D_MODEL, S), dtype=np.float32)
    for cix in range(N_CORES):
        outt += res.results[cix]["outt"]
    out = np.ascontiguousarray(outt.T)[None]  # [1, S, D]

    k_window = np.stack([res.results[cix]["ko"] for cix in range(N_CORES)])[None]
    v_window = np.stack([res.results[cix]["vo"] for cix in range(N_CORES)])[None]
    return out, k_window, v_window
